# revision 14
# baseline (speedup 1.0000x reference)
"""LLaMA-style MLP (gate/up/silu/down) on 8 Trainium2 NeuronCores.

Strategy: data-parallel over tokens (8192 tokens -> 1024/core), fp8
matmuls in DoubleRow perf mode (2 fp8 contraction elements per PE pass,
0.5 cycles/row -> 4x bf16 matmul throughput) with fp32 PSUM
accumulation and no collectives.

Accuracy is recovered with a TWO-pass "alpha-mix" error compensation
(vs the classic 3-pass hi/lo scheme): every operand V is encoded as

    V_hi = fp8(V)
    V_mx = fp8(sqrt(a)*V_hi + (V - V_hi)/sqrt(a)),   a = 1/8

and each logical matmul W@X becomes two fp8 DoubleRow matmuls
accumulated in PSUM:

    W_hi@X_hi + W_mx@X_mx
      = (1+a)*W_hi@X_hi + W_hi@X_lo + W_lo@X_hi + W_lo@X_lo/a + O(eps*sqrt(a))

so dividing the PSUM result by (1+a) recovers W@X with all first-order
quantization corrections included.  The residual error terms are the
a-suppressed re-quantization noise of the mix operands (~eps*sqrt(a)),
the a-fold overcounted lo*lo term (~eps^2/a) and the (1+a) under-scaled
corrections (~a*eps); at a=1/8 these balance to ~0.8% per matmul,
~1.3e-2 end to end (vs 2e-2 budget; the 3-pass scheme gave 2.8e-3 at
1.5x the PE cost).  PE cost: 2 passes * 0.25 cyc per 128x128 MAC tile
= 2/3 of the 3-pass scheme -> ~1.76 ms of matmul at 2.4 GHz.

Weights are pre-scaled by 64 on the host so their magnitudes sit in
e4m3's normal range; the scale and the 1/(1+a) correction are divided
back out on-device (SiLU input scale, down-proj output scale).

Layouts (host pre-permutes; partition dim first, contraction subtiles
paired for DoubleRow's [p, 2, free] operand shape):

  x   -> xh/x2 [n_tn, 128, D/128, TB]    x*[tn,p,k,t] = encoding of x[tok, k*128+p]
  Wg  -> wgh/wg2 [F/128, 128, D/128, 128]  = encoding of 64*Wg[fm*128+m, k*128+p]
  Wu  -> wuh/wu2 (same layout)
  Wd  -> wdh/wd2 [D/128, 128, F/128, 128]  = encoding of 64*Wd[dm*128+m, k*128+p]
  out <- y [D/128, 128, T] f32           y[dm,p,t] = out[tok, dm*128+p]

Per 512-token block: gate/up PSUM chains contract D in 16 pairs x 2
passes, SiLU(psg/(64(1+a))) on the scalar engine, then on the vector
engine hf = H/sqrt(a) (fused scale), hh = fp8(sqrt(a)*hf) and
h2 = fp8(hf + (sqrt(a)-1/sqrt(a))*hh), and the down projection
contracts F in 43 pairs x 2 passes.

Overlap details:
 - The first slab's chains are emitted pass-major (all (Wh,xh) chunks,
   then (Wmx,xmx)) with the warmup DMAs interleaved in the same order,
   so the PE starts as soon as Wh+xh land instead of waiting for all
   four operand tiles.
 - Dummy DoubleRow matmuls on memset scratch keep the PE continuously
   busy across the initial DMA window and the in-flight-x2 gap
   (N_WARM0/N_WARM1): the tensor engine's p-state ramp (half speed for
   3us after any idle) is spent on free work, never on real chains.
 - wd tiles are split into ~half-F (a/b) pieces so each transfer is
   ~2us and the ring-slot WAR frees mid-chain; down chains are emitted
   pass-major so they start once wdh_a lands.  The first down slab's
   pieces are prefetched into late-stage-A DMA slack.
 - The next token block's x tiles are prefetched in 8 k-chunks spread
   over the previous down phase (a single burst would head-of-line
   block that phase's wd transfers), then its first gate/up weight
   slab behind down slabs 9..12.
 - y is stored as bf16 (cast to f32 on the host): halves y DMA; adds
   ~1e-3 quantization noise, negligible in quadrature.
 - The kernel's final down chain is split into eight token-slice
   chains so y copies/stores overlap the remaining matmuls instead of
   trailing the kernel.

Measured: HW exec 1788362 ns (vs 2667680 ns for the 3-pass hi/lo
baseline), end-to-end relative error 1.34e-2 (budget 2e-2).
"""

import os
import sys

sys.path.insert(0, "/opt/trn_rl_repo")

import math
from contextlib import ExitStack

import numpy as np
import ml_dtypes

import concourse.bass as bass  # noqa: F401
import concourse.tile as tile
import concourse.mybir as mybir
from concourse import bacc
from concourse.bass_utils import run_bass_kernel_spmd

BF16 = mybir.dt.bfloat16
F32 = mybir.dt.float32
FP8 = mybir.dt.float8e4
NP_FP8 = ml_dtypes.float8_e4m3

# Problem shape (hardcoded per the task contract).
B, S, D, F = 4, 2048, 4096, 11008
N_CORES = 8
T_CORE = (B * S) // N_CORES  # tokens per core
TB = 512                     # token block (one PSUM bank of fp32)
W_SCALE = 64.0               # host-side weight scale (power of 2)
ALPHA = 0.125                # mix strength of the 2-pass compensation
SQA = math.sqrt(ALPHA)

# Warmup PE-priming: dummy DoubleRow matmuls on memset scratch keep the
# tensor engine continuously busy while the first DMAs land, so the p-state
# ramp (2x-slow first 3us after any idle) is spent on free work, not real
# chains.  Counts tuned against the timeline sim.
N_WARM0 = 280  # kernel start -> first real matmul
N_WARM1 = 48   # last-operand gap inside the first pass-major chain

DR = mybir.MatmulPerfMode.DoubleRow

LAST_RUN = {}


def build_module(T=T_CORE, tb=TB, d=D, f=F):
    """Build the single-core Bass module (same program on all 8 cores)."""
    n_tn = T // tb          # 2 token blocks
    n_dk = d // 128         # 32 contraction subtiles for gate/up
    n_fm = f // 128         # 86 F slabs
    n_fk = f // 128         # 86 contraction subtiles for down
    n_dm = d // 128         # 32 D slabs

    nc = bacc.Bacc("TRN2", target_bir_lowering=False, debug=False)
    xh = nc.dram_tensor("xh", [n_tn, 128, n_dk, tb], FP8, kind="ExternalInput").ap()
    x2 = nc.dram_tensor("x2", [n_tn, 128, n_dk, tb], FP8, kind="ExternalInput").ap()
    wgh = nc.dram_tensor("wgh", [n_fm, 128, n_dk, 128], FP8, kind="ExternalInput").ap()
    wg2 = nc.dram_tensor("wg2", [n_fm, 128, n_dk, 128], FP8, kind="ExternalInput").ap()
    wuh = nc.dram_tensor("wuh", [n_fm, 128, n_dk, 128], FP8, kind="ExternalInput").ap()
    wu2 = nc.dram_tensor("wu2", [n_fm, 128, n_dk, 128], FP8, kind="ExternalInput").ap()
    wdh = nc.dram_tensor("wdh", [n_dm, 128, n_fk, 128], FP8, kind="ExternalInput").ap()
    wd2 = nc.dram_tensor("wd2", [n_dm, 128, n_fk, 128], FP8, kind="ExternalInput").ap()
    y = nc.dram_tensor("y", [n_dm, 128, T], BF16, kind="ExternalOutput").ap()

    inv = 1.0 / (W_SCALE * (1.0 + ALPHA))  # undo weight scale + (1+a)

    with tile.TileContext(nc) as tc, ExitStack() as ctx:
        xpool = ctx.enter_context(tc.tile_pool(name="x", bufs=1))
        wpool = ctx.enter_context(tc.tile_pool(name="w", bufs=2))
        wdpool = ctx.enter_context(tc.tile_pool(name="wdp", bufs=2))
        hpool = ctx.enter_context(tc.tile_pool(name="h", bufs=1))
        spool = ctx.enter_context(tc.tile_pool(name="s", bufs=2))
        fpool = ctx.enter_context(tc.tile_pool(name="hf", bufs=2))
        ypool = ctx.enter_context(tc.tile_pool(name="y", bufs=2))
        psum = ctx.enter_context(tc.tile_pool(name="psum", bufs=4, space="PSUM"))
        psumy = ctx.enter_context(tc.tile_pool(name="psumy", bufs=2, space="PSUM"))
        zpool = ctx.enter_context(tc.tile_pool(name="z", bufs=1))

        n_pairs = n_dk // 2

        zw_sb = zpool.tile([128, 2, 128], FP8, tag="zw")
        nc.vector.memset(zw_sb[:], 0)
        zx_sb = zpool.tile([128, 2, 128], FP8, tag="zx")
        nc.vector.memset(zx_sb[:], 0)

        def emit_warm(n):
            """n small dummy DoubleRow matmuls into scratch PSUM (borrows a
            psy-tagged bank, which is otherwise idle during warmup)."""
            if n <= 0:
                return
            psz = psumy.tile([128, tb], F32, tag="psy")
            for i in range(n):
                nc.tensor.matmul(
                    psz[:, 0:128], zw_sb[:], zx_sb[:],
                    start=(i == 0), stop=(i == n - 1), perf_mode=DR,
                )

        def emit_proj(ps, wh_sb, w2_sb, xh_sb, x2_sb, pass_major, warm_fill=0):
            """One 2-pass DoubleRow accumulation chain into `ps`.

            chunk-major needs all 4 operand tiles before the first matmul;
            pass-major orders the groups (Wh,xh), (Wmx,xmx) so the chain can
            start as soon as Wh and xh have landed — used for the first
            slab, where the chain start gates on the initial DMAs.
            """
            groups = [(wh_sb, xh_sb), (w2_sb, x2_sb)]
            if pass_major:
                for gi, (w_sb, x_sb) in enumerate(groups):
                    if gi == 1:
                        # the last operand's transfer may still be in flight;
                        # busy-wait on scratch so the p-state ramp isn't reset
                        emit_warm(warm_fill)
                    for c in range(n_pairs):
                        sl = slice(2 * c, 2 * c + 2)
                        nc.tensor.matmul(
                            ps[:], w_sb[:, sl], x_sb[:, sl],
                            start=(gi == 0 and c == 0),
                            stop=(gi == 1 and c == n_pairs - 1),
                            perf_mode=DR,
                        )
            else:
                for c in range(n_pairs):
                    sl = slice(2 * c, 2 * c + 2)
                    nc.tensor.matmul(
                        ps[:], wh_sb[:, sl], xh_sb[:, sl],
                        start=(c == 0), stop=False, perf_mode=DR,
                    )
                    nc.tensor.matmul(
                        ps[:], w2_sb[:, sl], x2_sb[:, sl],
                        start=False, stop=(c == n_pairs - 1), perf_mode=DR,
                    )

        next_x = None
        next_w0 = None
        for tn in range(n_tn):
            first = tn == 0
            if first:
                # Interleave the first slab's weight DMAs with the x DMAs in
                # the order the pass-major chain consumes them.
                wgh_sb = wpool.tile([128, n_dk, 128], FP8, tag="wgh")
                nc.sync.dma_start(wgh_sb[:], wgh[0])
                xh_sb = xpool.tile([128, n_dk, tb], FP8, tag="xh")
                hk = n_dk // 2
                nc.sync.dma_start(xh_sb[:, :hk], xh[tn, :, :hk])
                nc.sync.dma_start(xh_sb[:, hk:], xh[tn, :, hk:])
                wg2_sb = wpool.tile([128, n_dk, 128], FP8, tag="wg2")
                nc.sync.dma_start(wg2_sb[:], wg2[0])
                x2_sb = xpool.tile([128, n_dk, tb], FP8, tag="x2")
                nc.sync.dma_start(x2_sb[:, :hk], x2[tn, :, :hk])
                nc.sync.dma_start(x2_sb[:, hk:], x2[tn, :, hk:])
                wuh_sb = wpool.tile([128, n_dk, 128], FP8, tag="wuh")
                nc.sync.dma_start(wuh_sb[:], wuh[0])
                wu2_sb = wpool.tile([128, n_dk, 128], FP8, tag="wu2")
                nc.sync.dma_start(wu2_sb[:], wu2[0])
                w0 = (wgh_sb, wg2_sb, wuh_sb, wu2_sb)
                emit_warm(N_WARM0)
            else:
                # Issued during the previous block's down projection so the
                # transfers overlap stage B instead of stalling this block.
                xh_sb, x2_sb = next_x
                w0 = next_w0

            hh_sb = hpool.tile([128, n_fm, tb], FP8, tag="hh")
            h2_sb = hpool.tile([128, n_fm, tb], FP8, tag="h2")

            # Stage A: gate/up projection + silu + mul + fp8 mix encoding,
            # one 128-row slab of F at a time.
            ka = 44
            next_wd0 = []
            for fm in range(n_fm):
                if fm in (60, 66, 72, 78):
                    # Prefetch the down phase's first wd slab into the DMA
                    # queue's late-stage-A idle slack, so stage B starts
                    # without a weight bubble.
                    i = (60, 66, 72, 78).index(fm)
                    wtag = ("wdh_a", "wdh_b", "wd2_a", "wd2_b")[i]
                    wsrc = (wdh, wdh, wd2, wd2)[i]
                    ksl = (slice(0, ka), slice(ka, n_fk))[i % 2]
                    nwd = wdpool.tile([128, ksl.stop - ksl.start, 128], FP8,
                                      tag=wtag)
                    nc.sync.dma_start(nwd[:], wsrc[0, :, ksl])
                    next_wd0.append(nwd)
                if fm == 0:
                    wgh_sb, wg2_sb, wuh_sb, wu2_sb = w0
                else:
                    wgh_sb = wpool.tile([128, n_dk, 128], FP8, tag="wgh")
                    nc.sync.dma_start(wgh_sb[:], wgh[fm])
                    wg2_sb = wpool.tile([128, n_dk, 128], FP8, tag="wg2")
                    nc.sync.dma_start(wg2_sb[:], wg2[fm])
                    wuh_sb = wpool.tile([128, n_dk, 128], FP8, tag="wuh")
                    nc.sync.dma_start(wuh_sb[:], wuh[fm])
                    wu2_sb = wpool.tile([128, n_dk, 128], FP8, tag="wu2")
                    nc.sync.dma_start(wu2_sb[:], wu2[fm])

                tm = first and fm == 0
                psg = psum.tile([128, tb], F32, tag="ps")
                emit_proj(psg, wgh_sb, wg2_sb, xh_sb, x2_sb, tm,
                          warm_fill=N_WARM1 if tm else 0)
                psu = psum.tile([128, tb], F32, tag="ps")
                emit_proj(psu, wuh_sb, wu2_sb, xh_sb, x2_sb, False)

                # sg = silu(psg * inv)  [bf16]
                sg = spool.tile([128, tb], BF16, tag="sg")
                nc.scalar.activation(
                    sg[:], psg[:], mybir.ActivationFunctionType.Silu, scale=inv
                )
                # hf = H/sqrt(a) = (psu*inv)*sg/sqrt(a)  [f32]
                hf = fpool.tile([128, tb], F32, tag="hf")
                nc.vector.scalar_tensor_tensor(
                    hf[:], psu[:], inv / SQA, sg[:],
                    mybir.AluOpType.mult, mybir.AluOpType.mult,
                )
                # hh = fp8(sqrt(a)*hf) = fp8(H)
                nc.vector.tensor_scalar_mul(hh_sb[:, fm], hf[:], SQA)
                # h2 = fp8(hf + (sqrt(a)-1/sqrt(a))*hh)
                #    = fp8(sqrt(a)*Hh + (H - Hh)/sqrt(a))
                nc.vector.scalar_tensor_tensor(
                    h2_sb[:, fm], hh_sb[:, fm], SQA - 1.0 / SQA, hf[:],
                    mybir.AluOpType.mult, mybir.AluOpType.add,
                )

            # Stage B: down projection, contracting over all of F.
            # wd tiles are split into a (44 ktiles) / b (42 ktiles) halves so
            # each DMA is ~2us and the ring-slot WAR dependency frees at the
            # reader's mid-chain instead of chain end — finer DMA pipelining.
            for dm in range(n_dm):
                if dm == 0:
                    wdh_a, wdh_b, wd2_a, wd2_b = next_wd0
                else:
                    wdh_a = wdpool.tile([128, ka, 128], FP8, tag="wdh_a")
                    nc.sync.dma_start(wdh_a[:], wdh[dm, :, :ka])
                    wdh_b = wdpool.tile([128, n_fk - ka, 128], FP8, tag="wdh_b")
                    nc.sync.dma_start(wdh_b[:], wdh[dm, :, ka:])
                    wd2_a = wdpool.tile([128, ka, 128], FP8, tag="wd2_a")
                    nc.sync.dma_start(wd2_a[:], wd2[dm, :, :ka])
                    wd2_b = wdpool.tile([128, n_fk - ka, 128], FP8, tag="wd2_b")
                    nc.sync.dma_start(wd2_b[:], wd2[dm, :, ka:])
                if tn + 1 < n_tn:
                    # Prefetch the next token block's x in 8 k-chunks spread
                    # over down slabs 1..8 (a single 4MB burst would
                    # head-of-line-block this stage's own wd transfers), then
                    # its first gate/up weight slab behind slabs 9..12 so the
                    # next stage-A starts without a DMA bubble.
                    if dm == 1:
                        nxh = xpool.tile([128, n_dk, tb], FP8, tag="xh")
                        nx2 = xpool.tile([128, n_dk, tb], FP8, tag="x2")
                        next_x = (nxh, nx2)
                    if 1 <= dm <= 8:
                        ck = n_dk // 8
                        ks = slice((dm - 1) * ck, dm * ck)
                        nc.sync.dma_start(next_x[0][:, ks], xh[tn + 1, :, ks])
                        nc.sync.dma_start(next_x[1][:, ks], x2[tn + 1, :, ks])
                    elif 9 <= dm <= 12:
                        wsrc = (wgh, wg2, wuh, wu2)[dm - 9]
                        wtag = ("wgh", "wg2", "wuh", "wu2")[dm - 9]
                        nw = wpool.tile([128, n_dk, 128], FP8, tag=wtag)
                        nc.sync.dma_start(nw[:], wsrc[0])
                        if dm == 9:
                            next_w0 = []
                        next_w0.append(nw)
                n_fpairs = n_fk // 2
                # Pass-major pair sequence over the split wd tiles: all
                # (wdh, hh) pairs first so the chain starts once wdh_a
                # lands, with wd2_* transfers still in flight.
                seq = []
                for wa, wb, h in ((wdh_a, wdh_b, hh_sb), (wd2_a, wd2_b, h2_sb)):
                    for c in range(n_fpairs):
                        g0 = 2 * c
                        if g0 + 2 <= ka:
                            w, wsl = wa, slice(g0, g0 + 2)
                        else:
                            w, wsl = wb, slice(g0 - ka, g0 - ka + 2)
                        seq.append((w, wsl, h, slice(g0, g0 + 2)))
                if tn == n_tn - 1 and dm == n_dm - 1:
                    # Last chain of the kernel: split into eight token-slice
                    # chains so earlier slices' y copy + store overlap the
                    # later slices' matmuls instead of trailing the kernel.
                    nsp = 8
                    for part in range(nsp):
                        ts = slice(part * (tb // nsp), (part + 1) * (tb // nsp))
                        psyh = psumy.tile([128, tb // nsp], F32, tag="psyh")
                        for i, (w, wsl, h, hsl) in enumerate(seq):
                            nc.tensor.matmul(
                                psyh[:], w[:, wsl], h[:, hsl, ts],
                                start=(i == 0), stop=(i == len(seq) - 1),
                                perf_mode=DR,
                            )
                        y_sb = ypool.tile([128, tb // nsp], BF16, tag="yh")
                        nc.vector.tensor_scalar_mul(y_sb[:], psyh[:], inv)
                        nc.sync.dma_start(
                            y[dm, :, tn * tb + part * (tb // nsp):
                                     tn * tb + (part + 1) * (tb // nsp)],
                            y_sb[:])
                    continue
                psy = psumy.tile([128, tb], F32, tag="psy")
                for i, (w, wsl, h, hsl) in enumerate(seq):
                    nc.tensor.matmul(
                        psy[:], w[:, wsl], h[:, hsl],
                        start=(i == 0), stop=(i == len(seq) - 1), perf_mode=DR,
                    )
                y_sb = ypool.tile([128, tb], BF16, tag="y")
                nc.vector.tensor_scalar_mul(y_sb[:], psy[:], inv)
                nc.sync.dma_start(y[dm, :, tn * tb:(tn + 1) * tb], y_sb[:])

    nc.compile()
    return nc


def _fp8_mix_enc(a):
    """Encode float32 array as (hi, mix) float8_e4m3 parts:
    hi = fp8(a), mix = fp8(sqrt(alpha)*hi + (a - hi)/sqrt(alpha))."""
    hi = a.astype(NP_FP8)
    hf = hi.astype(np.float32)
    mx = (np.float32(SQA) * hf + (a - hf) * np.float32(1.0 / SQA)).astype(NP_FP8)
    return hi, mx


def _prep_inputs(x, W_gate, W_up, W_down, T=T_CORE, tb=TB, d=D, f=F,
                 n_cores=N_CORES):
    """Host-side shard + permute + fp8 mix encoding. Returns in_maps."""
    n_tn = T // tb
    n_dk = d // 128
    n_fm = f // 128
    n_dm = d // 128

    tokens = np.ascontiguousarray(np.asarray(x, dtype=np.float32).reshape(-1, d))

    def perm_w(W, n_rows):
        # [n_rows*128, K] -> [n_rows, 128(p), K/128(k), 128(m)]
        return np.ascontiguousarray(
            W.reshape(n_rows, 128, -1, 128).transpose(0, 3, 2, 1))

    wg_hi, wg_mx = _fp8_mix_enc(np.asarray(W_gate, np.float32) * W_SCALE)
    wu_hi, wu_mx = _fp8_mix_enc(np.asarray(W_up, np.float32) * W_SCALE)
    wd_hi, wd_mx = _fp8_mix_enc(np.asarray(W_down, np.float32) * W_SCALE)

    wgh_np = perm_w(wg_hi, n_fm)
    wg2_np = perm_w(wg_mx, n_fm)
    wuh_np = perm_w(wu_hi, n_fm)
    wu2_np = perm_w(wu_mx, n_fm)
    wdh_np = perm_w(wd_hi, n_dm)
    wd2_np = perm_w(wd_mx, n_dm)

    in_maps = []
    for c in range(n_cores):
        xc = tokens[c * T:(c + 1) * T]  # [T, d]
        x_hi, x_mx = _fp8_mix_enc(xc)
        # [T, d] -> [n_tn, 128(p), n_dk(k), tb(t)]
        xh_np = np.ascontiguousarray(
            x_hi.reshape(n_tn, tb, n_dk, 128).transpose(0, 3, 2, 1))
        x2_np = np.ascontiguousarray(
            x_mx.reshape(n_tn, tb, n_dk, 128).transpose(0, 3, 2, 1))
        in_maps.append({
            "xh": xh_np, "x2": x2_np,
            "wgh": wgh_np, "wg2": wg2_np,
            "wuh": wuh_np, "wu2": wu2_np,
            "wdh": wdh_np, "wd2": wd2_np,
        })
    return in_maps


def _postprocess(results, T=T_CORE, d=D, n_cores=N_CORES):
    """y[dm, p, t] per core (bf16) -> full [B, S, D] float32."""
    outs = []
    for c in range(n_cores):
        yc = np.asarray(results[c]["y"]).astype(np.float32)  # [n_dm, 128, T]
        outs.append(yc.transpose(2, 0, 1).reshape(T, d))
    return np.concatenate(outs, axis=0)


def kernel(x, W_gate, W_up, W_down):
    import time

    if "nc" not in LAST_RUN:
        t0 = time.perf_counter()
        LAST_RUN["nc"] = build_module()
        LAST_RUN["build_s"] = time.perf_counter() - t0
    nc = LAST_RUN["nc"]

    t0 = time.perf_counter()
    in_maps = _prep_inputs(x, W_gate, W_up, W_down)
    LAST_RUN["prep_s"] = time.perf_counter() - t0

    t0 = time.perf_counter()
    res = run_bass_kernel_spmd(nc, in_maps, core_ids=list(range(N_CORES)))
    LAST_RUN["run_s"] = time.perf_counter() - t0
    LAST_RUN["results"] = res

    out = _postprocess(res.results)
    return out.reshape(B, S, D)


# revision 19
# speedup vs baseline: 1.0011x; 1.0011x over previous
"""LLaMA-style MLP (gate/up/silu/down) on 8 Trainium2 NeuronCores.

Strategy: data-parallel over tokens (8192 tokens -> 1024/core), fp8
matmuls in DoubleRow perf mode (2 fp8 contraction elements per PE pass,
0.5 cycles/row -> 4x bf16 matmul throughput) with fp32 PSUM
accumulation and no collectives.

Accuracy is recovered with a TWO-pass "alpha-mix" error compensation
(vs the classic 3-pass hi/lo scheme): every operand V is encoded as

    V_hi = fp8(V)
    V_mx = fp8(sqrt(a)*V_hi + (V - V_hi)/sqrt(a)),   a = 1/8

and each logical matmul W@X becomes two fp8 DoubleRow matmuls
accumulated in PSUM:

    W_hi@X_hi + W_mx@X_mx
      = (1+a)*W_hi@X_hi + W_hi@X_lo + W_lo@X_hi + W_lo@X_lo/a + O(eps*sqrt(a))

so dividing the PSUM result by (1+a) recovers W@X with all first-order
quantization corrections included.  The residual error terms are the
a-suppressed re-quantization noise of the mix operands (~eps*sqrt(a)),
the a-fold overcounted lo*lo term (~eps^2/a) and the (1+a) under-scaled
corrections (~a*eps); at a=1/8 these balance to ~0.8% per matmul,
~1.3e-2 end to end (vs 2e-2 budget; the 3-pass scheme gave 2.8e-3 at
1.5x the PE cost).  PE cost: 2 passes * 0.25 cyc per 128x128 MAC tile
= 2/3 of the 3-pass scheme -> ~1.76 ms of matmul at 2.4 GHz.

Weights are pre-scaled by 64 on the host so their magnitudes sit in
e4m3's normal range; the scale and the 1/(1+a) correction are divided
back out on-device (SiLU input scale, down-proj output scale).

Layouts (host pre-permutes; partition dim first, contraction subtiles
paired for DoubleRow's [p, 2, free] operand shape):

  x   -> xh/x2 [n_tn, 128, D/128, TB]    x*[tn,p,k,t] = encoding of x[tok, k*128+p]
  Wg  -> wgh/wg2 [F/128, 128, D/128, 128]  = encoding of 64*Wg[fm*128+m, k*128+p]
  Wu  -> wuh/wu2 (same layout)
  Wd  -> wdh/wd2 [D/128, 128, F/128, 128]  = encoding of 64*Wd[dm*128+m, k*128+p]
  out <- y [D/128, 128, T] f32           y[dm,p,t] = out[tok, dm*128+p]

Per 512-token block: gate/up PSUM chains contract D in 16 pairs x 2
passes, SiLU(psg/(64(1+a))) on the scalar engine, then on the vector
engine hf = H/sqrt(a) (fused scale), hh = fp8(sqrt(a)*hf) and
h2 = fp8(hf + (sqrt(a)-1/sqrt(a))*hh), and the down projection
contracts F in 43 pairs x 2 passes.

Overlap details:
 - The first slab's chains are emitted pass-major (all (Wh,xh) chunks,
   then (Wmx,xmx)) with the warmup DMAs interleaved in the same order,
   so the PE starts as soon as Wh+xh land instead of waiting for all
   four operand tiles.
 - Dummy DoubleRow matmuls on memset scratch keep the PE continuously
   busy across the initial DMA window and the in-flight-x2 gap
   (N_WARM0/N_WARM1): the tensor engine's p-state ramp (half speed for
   3us after any idle) is spent on free work, never on real chains.
 - wd tiles are split into ~half-F (a/b) pieces so each transfer is
   ~2us and the ring-slot WAR frees mid-chain; down chains are emitted
   pass-major so they start once wdh_a lands.  The first down slab's
   pieces are prefetched into late-stage-A DMA slack.
 - The next token block's x tiles are prefetched in 8 k-chunks spread
   over the previous down phase (a single burst would head-of-line
   block that phase's wd transfers), then its first gate/up weight
   slab behind down slabs 9..12.
 - y is stored as bf16 (cast to f32 on the host): halves y DMA; adds
   ~1e-3 quantization noise, negligible in quadrature.
 - The kernel's final down chain is split into eight token-slice
   chains so y copies/stores overlap the remaining matmuls instead of
   trailing the kernel.

Measured: HW exec 1788362 ns (vs 2667680 ns for the 3-pass hi/lo
baseline), end-to-end relative error 1.34e-2 (budget 2e-2).
"""

import os
import sys

sys.path.insert(0, "/opt/trn_rl_repo")

import math
from contextlib import ExitStack

import numpy as np
import ml_dtypes

import concourse.bass as bass  # noqa: F401
import concourse.tile as tile
import concourse.mybir as mybir
from concourse import bacc
from concourse.bass_utils import run_bass_kernel_spmd

BF16 = mybir.dt.bfloat16
F32 = mybir.dt.float32
FP8 = mybir.dt.float8e4
NP_FP8 = ml_dtypes.float8_e4m3

# Problem shape (hardcoded per the task contract).
B, S, D, F = 4, 2048, 4096, 11008
N_CORES = 8
T_CORE = (B * S) // N_CORES  # tokens per core
TB = 512                     # token block (one PSUM bank of fp32)
W_SCALE = 64.0               # host-side weight scale (power of 2)
ALPHA = 0.125                # mix strength of the 2-pass compensation
SQA = math.sqrt(ALPHA)

# Warmup PE-priming: dummy DoubleRow matmuls on memset scratch keep the
# tensor engine continuously busy while the first DMAs land, so the p-state
# ramp (2x-slow first 3us after any idle) is spent on free work, not real
# chains.  Counts tuned against the timeline sim.
N_WARM0 = 280  # kernel start -> first real matmul
N_WARM1 = 48   # last-operand gap inside the first pass-major chain

DR = mybir.MatmulPerfMode.DoubleRow

LAST_RUN = {}


def build_module(T=T_CORE, tb=TB, d=D, f=F):
    """Build the single-core Bass module (same program on all 8 cores)."""
    n_tn = T // tb          # 2 token blocks
    n_dk = d // 128         # 32 contraction subtiles for gate/up
    n_fm = f // 128         # 86 F slabs
    n_fk = f // 128         # 86 contraction subtiles for down
    n_dm = d // 128         # 32 D slabs

    nc = bacc.Bacc("TRN2", target_bir_lowering=False, debug=False)
    xh = nc.dram_tensor("xh", [n_tn, 128, n_dk, tb], FP8, kind="ExternalInput").ap()
    x2 = nc.dram_tensor("x2", [n_tn, 128, n_dk, tb], FP8, kind="ExternalInput").ap()
    wgh = nc.dram_tensor("wgh", [n_fm, 128, n_dk, 128], FP8, kind="ExternalInput").ap()
    wg2 = nc.dram_tensor("wg2", [n_fm, 128, n_dk, 128], FP8, kind="ExternalInput").ap()
    wuh = nc.dram_tensor("wuh", [n_fm, 128, n_dk, 128], FP8, kind="ExternalInput").ap()
    wu2 = nc.dram_tensor("wu2", [n_fm, 128, n_dk, 128], FP8, kind="ExternalInput").ap()
    wdh = nc.dram_tensor("wdh", [n_dm, 128, n_fk, 128], FP8, kind="ExternalInput").ap()
    wd2 = nc.dram_tensor("wd2", [n_dm, 128, n_fk, 128], FP8, kind="ExternalInput").ap()
    y = nc.dram_tensor("y", [n_dm, 128, T], BF16, kind="ExternalOutput").ap()

    inv = 1.0 / (W_SCALE * (1.0 + ALPHA))  # undo weight scale + (1+a)

    with tile.TileContext(nc) as tc, ExitStack() as ctx:
        xpool = ctx.enter_context(tc.tile_pool(name="x", bufs=1))
        wpool = ctx.enter_context(tc.tile_pool(name="w", bufs=2))
        wdpool = ctx.enter_context(tc.tile_pool(name="wdp", bufs=2))
        hpool = ctx.enter_context(tc.tile_pool(name="h", bufs=1))
        spool = ctx.enter_context(tc.tile_pool(name="s", bufs=2))
        fpool = ctx.enter_context(tc.tile_pool(name="hf", bufs=2))
        ypool = ctx.enter_context(tc.tile_pool(name="y", bufs=2))
        psum = ctx.enter_context(tc.tile_pool(name="psum", bufs=4, space="PSUM"))
        psumy = ctx.enter_context(tc.tile_pool(name="psumy", bufs=2, space="PSUM"))
        zpool = ctx.enter_context(tc.tile_pool(name="z", bufs=1))

        n_pairs = n_dk // 2

        zw_sb = zpool.tile([128, 2, 128], FP8, tag="zw")
        nc.vector.memset(zw_sb[:], 0)
        zx_sb = zpool.tile([128, 2, 128], FP8, tag="zx")
        nc.vector.memset(zx_sb[:], 0)

        def emit_warm(n):
            """n small dummy DoubleRow matmuls into scratch PSUM (borrows a
            psy-tagged bank, which is otherwise idle during warmup)."""
            if n <= 0:
                return
            psz = psumy.tile([128, tb], F32, tag="psy")
            for i in range(n):
                nc.tensor.matmul(
                    psz[:, 0:128], zw_sb[:], zx_sb[:],
                    start=(i == 0), stop=(i == n - 1), perf_mode=DR,
                )

        def emit_proj(ps, wh_sb, w2_sb, xh_sb, x2_sb, pass_major, warm_fill=0):
            """One 2-pass DoubleRow accumulation chain into `ps`.

            chunk-major needs all 4 operand tiles before the first matmul;
            pass-major orders the groups (Wh,xh), (Wmx,xmx) so the chain can
            start as soon as Wh and xh have landed — used for the first
            slab, where the chain start gates on the initial DMAs.
            """
            groups = [(wh_sb, xh_sb), (w2_sb, x2_sb)]
            if pass_major:
                for gi, (w_sb, x_sb) in enumerate(groups):
                    if gi == 1:
                        # the last operand's transfer may still be in flight;
                        # busy-wait on scratch so the p-state ramp isn't reset
                        emit_warm(warm_fill)
                    for c in range(n_pairs):
                        sl = slice(2 * c, 2 * c + 2)
                        nc.tensor.matmul(
                            ps[:], w_sb[:, sl], x_sb[:, sl],
                            start=(gi == 0 and c == 0),
                            stop=(gi == 1 and c == n_pairs - 1),
                            perf_mode=DR,
                        )
            else:
                for c in range(n_pairs):
                    sl = slice(2 * c, 2 * c + 2)
                    nc.tensor.matmul(
                        ps[:], wh_sb[:, sl], xh_sb[:, sl],
                        start=(c == 0), stop=False, perf_mode=DR,
                    )
                    nc.tensor.matmul(
                        ps[:], w2_sb[:, sl], x2_sb[:, sl],
                        start=False, stop=(c == n_pairs - 1), perf_mode=DR,
                    )

        next_x = None
        next_w0 = None
        for tn in range(n_tn):
            first = tn == 0
            if first:
                # Interleave the first slab's weight DMAs with the x DMAs in
                # the order the pass-major chain consumes them.
                wgh_sb = wpool.tile([128, n_dk, 128], FP8, tag="wgh")
                nc.sync.dma_start(wgh_sb[:], wgh[0])
                xh_sb = xpool.tile([128, n_dk, tb], FP8, tag="xh")
                hk = n_dk // 2
                nc.sync.dma_start(xh_sb[:, :hk], xh[tn, :, :hk])
                nc.sync.dma_start(xh_sb[:, hk:], xh[tn, :, hk:])
                wuh_sb = wpool.tile([128, n_dk, 128], FP8, tag="wuh")
                nc.sync.dma_start(wuh_sb[:], wuh[0])
                wg2_sb = wpool.tile([128, n_dk, 128], FP8, tag="wg2")
                nc.sync.dma_start(wg2_sb[:], wg2[0])
                x2_sb = xpool.tile([128, n_dk, tb], FP8, tag="x2")
                nc.sync.dma_start(x2_sb[:, :hk], x2[tn, :, :hk])
                nc.sync.dma_start(x2_sb[:, hk:], x2[tn, :, hk:])
                wu2_sb = wpool.tile([128, n_dk, 128], FP8, tag="wu2")
                nc.sync.dma_start(wu2_sb[:], wu2[0])
                w0 = (wgh_sb, wg2_sb, wuh_sb, wu2_sb)
                emit_warm(N_WARM0)
            else:
                # Issued during the previous block's down projection so the
                # transfers overlap stage B instead of stalling this block.
                xh_sb, x2_sb = next_x
                w0 = next_w0

            hh_sb = hpool.tile([128, n_fm, tb], FP8, tag="hh")
            h2_sb = hpool.tile([128, n_fm, tb], FP8, tag="h2")

            # Stage A: gate/up projection + silu + mul + fp8 mix encoding,
            # one 128-row slab of F at a time.
            ka = 44
            next_wd0 = []
            for fm in range(n_fm):
                if fm in (60, 66, 72, 78):
                    # Prefetch the down phase's first wd slab into the DMA
                    # queue's late-stage-A idle slack, so stage B starts
                    # without a weight bubble.
                    i = (60, 66, 72, 78).index(fm)
                    wtag = ("wdh_a", "wdh_b", "wd2_a", "wd2_b")[i]
                    wsrc = (wdh, wdh, wd2, wd2)[i]
                    ksl = (slice(0, ka), slice(ka, n_fk))[i % 2]
                    nwd = wdpool.tile([128, ksl.stop - ksl.start, 128], FP8,
                                      tag=wtag)
                    nc.sync.dma_start(nwd[:], wsrc[0, :, ksl])
                    next_wd0.append(nwd)
                if fm == 0:
                    wgh_sb, wg2_sb, wuh_sb, wu2_sb = w0
                else:
                    wgh_sb = wpool.tile([128, n_dk, 128], FP8, tag="wgh")
                    nc.sync.dma_start(wgh_sb[:], wgh[fm])
                    wg2_sb = wpool.tile([128, n_dk, 128], FP8, tag="wg2")
                    nc.sync.dma_start(wg2_sb[:], wg2[fm])
                    wuh_sb = wpool.tile([128, n_dk, 128], FP8, tag="wuh")
                    nc.sync.dma_start(wuh_sb[:], wuh[fm])
                    wu2_sb = wpool.tile([128, n_dk, 128], FP8, tag="wu2")
                    nc.sync.dma_start(wu2_sb[:], wu2[fm])

                tm = first and fm == 0
                psg = psum.tile([128, tb], F32, tag="ps")
                psu = psum.tile([128, tb], F32, tag="ps")
                if tm:
                    # Hi passes of BOTH projections first (their operands
                    # land first: wgh, xh, wuh), then warm over the in-flight
                    # x2 window, then both mix passes.  Both PSUM chains stay
                    # open across the interleave (separate banks).
                    for ps, w_sb, x_sb, st, sp in (
                        (psg, wgh_sb, xh_sb, True, False),
                        (psu, wuh_sb, xh_sb, True, False),
                        (None, None, None, None, None),
                        (psg, wg2_sb, x2_sb, False, True),
                        (psu, wu2_sb, x2_sb, False, True),
                    ):
                        if ps is None:
                            emit_warm(N_WARM1)
                            continue
                        for c in range(n_pairs):
                            sl = slice(2 * c, 2 * c + 2)
                            nc.tensor.matmul(
                                ps[:], w_sb[:, sl], x_sb[:, sl],
                                start=(st and c == 0),
                                stop=(sp and c == n_pairs - 1),
                                perf_mode=DR,
                            )
                else:
                    emit_proj(psg, wgh_sb, wg2_sb, xh_sb, x2_sb, False)
                    emit_proj(psu, wuh_sb, wu2_sb, xh_sb, x2_sb, False)

                # sg = silu(psg * inv)  [bf16]
                sg = spool.tile([128, tb], BF16, tag="sg")
                nc.scalar.activation(
                    sg[:], psg[:], mybir.ActivationFunctionType.Silu, scale=inv
                )
                # hf = H/sqrt(a) = (psu*inv)*sg/sqrt(a)  [f32]
                hf = fpool.tile([128, tb], F32, tag="hf")
                nc.vector.scalar_tensor_tensor(
                    hf[:], psu[:], inv / SQA, sg[:],
                    mybir.AluOpType.mult, mybir.AluOpType.mult,
                )
                # hh = fp8(sqrt(a)*hf) = fp8(H)
                nc.vector.tensor_scalar_mul(hh_sb[:, fm], hf[:], SQA)
                # h2 = fp8(hf + (sqrt(a)-1/sqrt(a))*hh)
                #    = fp8(sqrt(a)*Hh + (H - Hh)/sqrt(a))
                nc.vector.scalar_tensor_tensor(
                    h2_sb[:, fm], hh_sb[:, fm], SQA - 1.0 / SQA, hf[:],
                    mybir.AluOpType.mult, mybir.AluOpType.add,
                )

            # Stage B: down projection, contracting over all of F.
            # wd tiles are split into a (44 ktiles) / b (42 ktiles) halves so
            # each DMA is ~2us and the ring-slot WAR dependency frees at the
            # reader's mid-chain instead of chain end — finer DMA pipelining.
            for dm in range(n_dm):
                if dm == 0:
                    wdh_a, wdh_b, wd2_a, wd2_b = next_wd0
                else:
                    wdh_a = wdpool.tile([128, ka, 128], FP8, tag="wdh_a")
                    nc.sync.dma_start(wdh_a[:], wdh[dm, :, :ka])
                    wdh_b = wdpool.tile([128, n_fk - ka, 128], FP8, tag="wdh_b")
                    nc.sync.dma_start(wdh_b[:], wdh[dm, :, ka:])
                    wd2_a = wdpool.tile([128, ka, 128], FP8, tag="wd2_a")
                    nc.sync.dma_start(wd2_a[:], wd2[dm, :, :ka])
                    wd2_b = wdpool.tile([128, n_fk - ka, 128], FP8, tag="wd2_b")
                    nc.sync.dma_start(wd2_b[:], wd2[dm, :, ka:])
                if tn + 1 < n_tn:
                    # Prefetch the next token block's x in 8 k-chunks spread
                    # over down slabs 1..8 (a single 4MB burst would
                    # head-of-line-block this stage's own wd transfers), then
                    # its first gate/up weight slab behind slabs 9..12 so the
                    # next stage-A starts without a DMA bubble.
                    if dm == 1:
                        nxh = xpool.tile([128, n_dk, tb], FP8, tag="xh")
                        nx2 = xpool.tile([128, n_dk, tb], FP8, tag="x2")
                        next_x = (nxh, nx2)
                    if 1 <= dm <= 8:
                        ck = n_dk // 8
                        ks = slice((dm - 1) * ck, dm * ck)
                        nc.sync.dma_start(next_x[0][:, ks], xh[tn + 1, :, ks])
                        nc.sync.dma_start(next_x[1][:, ks], x2[tn + 1, :, ks])
                    elif 9 <= dm <= 12:
                        wsrc = (wgh, wg2, wuh, wu2)[dm - 9]
                        wtag = ("wgh", "wg2", "wuh", "wu2")[dm - 9]
                        nw = wpool.tile([128, n_dk, 128], FP8, tag=wtag)
                        nc.sync.dma_start(nw[:], wsrc[0])
                        if dm == 9:
                            next_w0 = []
                        next_w0.append(nw)
                n_fpairs = n_fk // 2
                # Pass-major pair sequence over the split wd tiles: all
                # (wdh, hh) pairs first so the chain starts once wdh_a
                # lands, with wd2_* transfers still in flight.
                seq = []
                for wa, wb, h in ((wdh_a, wdh_b, hh_sb), (wd2_a, wd2_b, h2_sb)):
                    for c in range(n_fpairs):
                        g0 = 2 * c
                        if g0 + 2 <= ka:
                            w, wsl = wa, slice(g0, g0 + 2)
                        else:
                            w, wsl = wb, slice(g0 - ka, g0 - ka + 2)
                        seq.append((w, wsl, h, slice(g0, g0 + 2)))
                if tn == n_tn - 1 and dm == n_dm - 1:
                    # Last chain of the kernel: split into eight token-slice
                    # chains so earlier slices' y copy + store overlap the
                    # later slices' matmuls instead of trailing the kernel.
                    nsp = 8
                    for part in range(nsp):
                        ts = slice(part * (tb // nsp), (part + 1) * (tb // nsp))
                        psyh = psumy.tile([128, tb // nsp], F32, tag="psyh")
                        for i, (w, wsl, h, hsl) in enumerate(seq):
                            nc.tensor.matmul(
                                psyh[:], w[:, wsl], h[:, hsl, ts],
                                start=(i == 0), stop=(i == len(seq) - 1),
                                perf_mode=DR,
                            )
                        y_sb = ypool.tile([128, tb // nsp], BF16, tag="yh", bufs=6)
                        nc.vector.tensor_scalar_mul(y_sb[:], psyh[:], inv)
                        nc.sync.dma_start(
                            y[dm, :, tn * tb + part * (tb // nsp):
                                     tn * tb + (part + 1) * (tb // nsp)],
                            y_sb[:])
                    continue
                psy = psumy.tile([128, tb], F32, tag="psy")
                for i, (w, wsl, h, hsl) in enumerate(seq):
                    nc.tensor.matmul(
                        psy[:], w[:, wsl], h[:, hsl],
                        start=(i == 0), stop=(i == len(seq) - 1), perf_mode=DR,
                    )
                y_sb = ypool.tile([128, tb], BF16, tag="y", bufs=3)
                nc.vector.tensor_scalar_mul(y_sb[:], psy[:], inv)
                nc.sync.dma_start(y[dm, :, tn * tb:(tn + 1) * tb], y_sb[:])

    nc.compile()
    return nc


def _fp8_mix_enc(a):
    """Encode float32 array as (hi, mix) float8_e4m3 parts:
    hi = fp8(a), mix = fp8(sqrt(alpha)*hi + (a - hi)/sqrt(alpha))."""
    hi = a.astype(NP_FP8)
    hf = hi.astype(np.float32)
    mx = (np.float32(SQA) * hf + (a - hf) * np.float32(1.0 / SQA)).astype(NP_FP8)
    return hi, mx


def _prep_inputs(x, W_gate, W_up, W_down, T=T_CORE, tb=TB, d=D, f=F,
                 n_cores=N_CORES):
    """Host-side shard + permute + fp8 mix encoding. Returns in_maps."""
    n_tn = T // tb
    n_dk = d // 128
    n_fm = f // 128
    n_dm = d // 128

    tokens = np.ascontiguousarray(np.asarray(x, dtype=np.float32).reshape(-1, d))

    def perm_w(W, n_rows):
        # [n_rows*128, K] -> [n_rows, 128(p), K/128(k), 128(m)]
        return np.ascontiguousarray(
            W.reshape(n_rows, 128, -1, 128).transpose(0, 3, 2, 1))

    wg_hi, wg_mx = _fp8_mix_enc(np.asarray(W_gate, np.float32) * W_SCALE)
    wu_hi, wu_mx = _fp8_mix_enc(np.asarray(W_up, np.float32) * W_SCALE)
    wd_hi, wd_mx = _fp8_mix_enc(np.asarray(W_down, np.float32) * W_SCALE)

    wgh_np = perm_w(wg_hi, n_fm)
    wg2_np = perm_w(wg_mx, n_fm)
    wuh_np = perm_w(wu_hi, n_fm)
    wu2_np = perm_w(wu_mx, n_fm)
    wdh_np = perm_w(wd_hi, n_dm)
    wd2_np = perm_w(wd_mx, n_dm)

    in_maps = []
    for c in range(n_cores):
        xc = tokens[c * T:(c + 1) * T]  # [T, d]
        x_hi, x_mx = _fp8_mix_enc(xc)
        # [T, d] -> [n_tn, 128(p), n_dk(k), tb(t)]
        xh_np = np.ascontiguousarray(
            x_hi.reshape(n_tn, tb, n_dk, 128).transpose(0, 3, 2, 1))
        x2_np = np.ascontiguousarray(
            x_mx.reshape(n_tn, tb, n_dk, 128).transpose(0, 3, 2, 1))
        in_maps.append({
            "xh": xh_np, "x2": x2_np,
            "wgh": wgh_np, "wg2": wg2_np,
            "wuh": wuh_np, "wu2": wu2_np,
            "wdh": wdh_np, "wd2": wd2_np,
        })
    return in_maps


def _postprocess(results, T=T_CORE, d=D, n_cores=N_CORES):
    """y[dm, p, t] per core (bf16) -> full [B, S, D] float32."""
    outs = []
    for c in range(n_cores):
        yc = np.asarray(results[c]["y"]).astype(np.float32)  # [n_dm, 128, T]
        outs.append(yc.transpose(2, 0, 1).reshape(T, d))
    return np.concatenate(outs, axis=0)


def kernel(x, W_gate, W_up, W_down):
    import time

    if "nc" not in LAST_RUN:
        t0 = time.perf_counter()
        LAST_RUN["nc"] = build_module()
        LAST_RUN["build_s"] = time.perf_counter() - t0
    nc = LAST_RUN["nc"]

    t0 = time.perf_counter()
    in_maps = _prep_inputs(x, W_gate, W_up, W_down)
    LAST_RUN["prep_s"] = time.perf_counter() - t0

    t0 = time.perf_counter()
    res = run_bass_kernel_spmd(nc, in_maps, core_ids=list(range(N_CORES)))
    LAST_RUN["run_s"] = time.perf_counter() - t0
    LAST_RUN["results"] = res

    out = _postprocess(res.results)
    return out.reshape(B, S, D)
